# revision 7
# baseline (speedup 1.0000x reference)
"""Tied-row (MSA) attention, sharded over 8 TRN2 NeuronCores.

Reference computation (b=1, r=128 MSA rows, n=512, 8 heads x 64):
    q, k, v = x @ Wq, x @ Wk, x @ Wv          per-row projections
    dots[h,i,j] = sum_{r,d} q[r,h,i,d] k[r,h,j,d] * scale / sqrt(num_rows)
    attn = softmax_j(dots)                     shared across rows
    out[r,i] = (sum_j attn[h,i,j] v[r,h,j,d]) @ Wo + bo

Sharding: MSA-row axis r split 16-per-core.  Each core computes its partial
logits (reduction over its local r), the 8 partials are summed with a bf16
ReduceScatter (each core then owns one head's logits = its rank), softmaxed
and transposed locally, and AllGathered back as attn^T.  Everything else
(projections, attn @ v, output projection) is core-local.

x^T / v / attn are bf16 on SBUF (capacity + collective size); q^T/k^T and
out^T are fp32 with float32r (full-rate) matmuls; accumulation is always
fp32 in PSUM.
"""

import numpy as np

import concourse.bacc as bacc
import concourse.bass as bass
import concourse.mybir as mybir
import concourse.tile as tile
from concourse import bass_utils
from concourse.masks import make_identity

CORES = 8
R = 16          # MSA rows per core
N = 512         # sequence length
DIM = 256       # model dim
H = 8           # heads
D = 64          # head dim
HD = H * D      # 512
RN = R * N      # 8192 token-rows per core

F32 = mybir.dt.float32
F32R = mybir.dt.float32r
BF16 = mybir.dt.bfloat16

RG = [list(range(CORES))]


def build_nc(scale: float):
    nc = bacc.Bacc(None, target_bir_lowering=False, debug=False)

    x_ext = nc.declare_dram_parameter("x", [RN, DIM], F32, isOutput=False)
    wq_ext = nc.declare_dram_parameter("wq", [DIM, HD], F32, isOutput=False)
    wk_ext = nc.declare_dram_parameter("wk", [DIM, HD], F32, isOutput=False)
    wv_ext = nc.declare_dram_parameter("wv", [DIM, HD], F32, isOutput=False)
    wo_ext = nc.declare_dram_parameter("wo", [HD, DIM], F32, isOutput=False)
    out_ext = nc.declare_dram_parameter("out", [RN, DIM], F32, isOutput=True)

    with tile.TileContext(nc) as tc:
        # ---- DRAM bounce buffers for the collectives ----
        dram = tc.alloc_tile_pool(name="dram", bufs=1, space="DRAM")
        rs_in = dram.tile([H * N, N], BF16, tag="rs_in")
        rs_out = dram.tile([N, N], BF16, tag="rs_out")
        ag_in = dram.tile([N, N], BF16, tag="ag_in")
        ag_out = dram.tile([H * N, N], BF16, tag="ag_out", addr_space="Shared")

        # ---- constants ----
        consts = tc.alloc_tile_pool(name="consts", bufs=1)
        wq_sb = consts.tile([128, 2, HD], BF16, tag="wq")
        wk_sb = consts.tile([128, 2, HD], BF16, tag="wk")
        wv_sb = consts.tile([128, 2, HD], BF16, tag="wv")
        wqf = consts.tile([128, 2, HD], F32, tag="wqf")
        wo_sb = consts.tile([128, 4, DIM], F32, tag="wo")
        wo_r = consts.tile([128, 4, DIM], F32R, tag="wor")
        id32 = consts.tile([128, 128], F32, tag="id32")
        idbf = consts.tile([128, 128], BF16, tag="idbf")
        # f32 weights come in via one staging tile, cast to bf16 on DVE
        for wext, wsb in ((wq_ext, wq_sb), (wk_ext, wk_sb), (wv_ext, wv_sb)):
            nc.sync.dma_start(
                out=wqf[:], in_=wext[:, :].rearrange("(k p) n -> p k n", p=128)
            )
            nc.any.tensor_copy(wsb[:], wqf[:])
        nc.sync.dma_start(
            out=wo_sb[:], in_=wo_ext[:, :].rearrange("(k p) n -> p k n", p=128)
        )
        nc.any.tensor_copy(wo_r[:], wo_sb[:])
        make_identity(nc, id32[:])
        make_identity(nc, idbf[:])

        # v tiles live from the v-projection (overlapping the ReduceScatter)
        # until attn @ v in phase B, so the pool outlives xT -> allocate first
        v_pool = tc.alloc_tile_pool(name="v", bufs=R * 4)
        xT_pool = tc.alloc_tile_pool(name="xT", bufs=1)
        xT = xT_pool.tile([128, 2, RN], BF16, tag="xT")

        xp_psum = tc.alloc_tile_pool(name="xp_psum", bufs=2, space="PSUM")
        proj_psum = tc.alloc_tile_pool(name="proj_psum", bufs=2, space="PSUM")
        dots_psum = tc.alloc_tile_pool(name="dots_psum", bufs=4, space="PSUM")

        # ---- load x and transpose to x^T [dim(2x128), rn] (bf16) ----
        xrow_pool = tc.alloc_tile_pool(name="xrow", bufs=4)
        for c in range(RN // 128):
            xr = xrow_pool.tile([128, DIM], F32, tag="xr")
            nc.sync.dma_start(out=xr[:], in_=x_ext[c * 128:(c + 1) * 128, :])
            for kc in range(2):
                pt = xp_psum.tile([128, 128], F32, tag="xp")
                nc.tensor.transpose(pt[:], xr[:, kc * 128:(kc + 1) * 128], id32[:])
                nc.any.tensor_copy(xT[:, kc, c * 128:(c + 1) * 128], pt[:])
        xrow_pool.release()

        # ---- per head-pair: project q^T, k^T and accumulate partial logits ----
        qkT_pool = tc.alloc_tile_pool(name="qkT", bufs=1)
        dstage_pool = tc.alloc_tile_pool(name="dstage", bufs=6)
        for hp in range(4):
            qT = qkT_pool.tile([128, RN], F32R, tag="qT")
            kT = qkT_pool.tile([128, RN], F32R, tag="kT")
            for wsb, dstT in ((wq_sb, qT), (wk_sb, kT)):
                for ch in range(RN // N):
                    ps = proj_psum.tile([128, N], F32, tag="proj")
                    for kc in range(2):
                        nc.tensor.matmul(
                            ps[:],
                            wsb[:, kc, hp * 128:(hp + 1) * 128],
                            xT[:, kc, ch * N:(ch + 1) * N],
                            start=(kc == 0),
                            stop=(kc == 1),
                        )
                    nc.any.tensor_copy(dstT[:, ch * N:(ch + 1) * N], ps[:])

            # partial dots for the two heads of this pair; the even head uses
            # PE row-group 0-63, the odd head 64-127 (concurrent row tiles)
            for ic in range(4):
                pe_ = dots_psum.tile([128, N], F32, tag="dots")
                po_ = dots_psum.tile([128, N], F32, tag="dots")
                for rr in range(R):
                    base = rr * N
                    isl = slice(base + ic * 128, base + ic * 128 + 128)
                    jsl = slice(base, base + N)
                    nc.tensor.matmul(
                        pe_[:],
                        qT[0:64, isl],
                        kT[0:64, jsl],
                        start=(rr == 0),
                        stop=(rr == R - 1),
                        skip_group_check=True,
                    )
                    nc.tensor.matmul(
                        po_[:],
                        qT[64:128, isl],
                        kT[64:128, jsl],
                        start=(rr == 0),
                        stop=(rr == R - 1),
                        skip_group_check=True,
                    )
                for m, ps in ((0, pe_), (1, po_)):
                    h = 2 * hp + m
                    st = dstage_pool.tile([128, N], BF16, tag="dstage")
                    nc.any.tensor_copy(st[:], ps[:])
                    row0 = h * N + ic * 128
                    nc.sync.dma_start(out=rs_in[row0:row0 + 128, :], in_=st[:])

        dstage_pool.release()
        qkT_pool.release()
        dots_psum.release()

        # ---- sum partial logits across cores; rank owns head==rank ----
        nc.gpsimd.collective_compute(
            "ReduceScatter",
            mybir.AluOpType.add,
            replica_groups=RG,
            ins=[rs_in[:, :].opt()],
            outs=[rs_out[:, :].opt()],
        )

        # ---- v projection (overlaps the collective; last use of xT) ----
        v_tiles = {}
        for rr in range(R):
            for jc in range(4):
                ps = proj_psum.tile([128, N], F32, tag="proj")
                for kc in range(2):
                    nc.tensor.matmul(
                        ps[:],
                        xT[:, kc, rr * N + jc * 128:rr * N + jc * 128 + 128],
                        wv_sb[:, kc, :],
                        start=(kc == 0),
                        stop=(kc == 1),
                    )
                vt = v_pool.tile([128, HD], BF16, tag="v")
                nc.any.tensor_copy(vt[:], ps[:])
                v_tiles[(rr, jc)] = vt
        xT_pool.release()
        proj_psum.release()

        # ---- softmax of the owned head, then transpose -> attn^T ----
        atp_psum = tc.alloc_tile_pool(name="atp_psum", bufs=2, space="PSUM")
        smax_pool = tc.alloc_tile_pool(name="smax", bufs=4)
        attnbf = []
        for ic in range(4):
            zt = smax_pool.tile([128, N], BF16, tag="zt")
            nc.sync.dma_start(out=zt[:], in_=rs_out[ic * 128:(ic + 1) * 128, :])
            att_f = smax_pool.tile([128, N], F32, tag="att_f")
            sums = smax_pool.tile([128, 1], F32, tag="sums")
            nc.scalar.activation(
                att_f[:],
                zt[:],
                mybir.ActivationFunctionType.Exp,
                scale=scale,
                accum_out=sums[:],
            )
            recip = smax_pool.tile([128, 1], F32, tag="recip")
            nc.vector.reciprocal(recip[:], sums[:])
            abf = smax_pool.tile([128, N], BF16, tag="abf")
            nc.vector.tensor_scalar_mul(abf[:], att_f[:], recip[:])
            attnbf.append(abf)
        for jc in range(4):
            ast = smax_pool.tile([128, N], BF16, tag="agst")
            for ic in range(4):
                pt = atp_psum.tile([128, 128], BF16, tag="atp")
                nc.tensor.transpose(pt[:], attnbf[ic][:, jc * 128:(jc + 1) * 128], idbf[:])
                nc.any.tensor_copy(ast[:, ic * 128:(ic + 1) * 128], pt[:])
            nc.sync.dma_start(out=ag_in[jc * 128:(jc + 1) * 128, :], in_=ast[:])

        nc.gpsimd.collective_compute(
            "AllGather",
            mybir.AluOpType.bypass,
            replica_groups=RG,
            ins=[ag_in[:, :].opt()],
            outs=[ag_out[:, :].opt()],
        )
        smax_pool.release()
        atp_psum.release()
        xp_psum.release()

        # ---- attn^T @ v -> out^T, then out @ Wo ----
        attnT_pool = tc.alloc_tile_pool(name="attnT", bufs=1)
        oT_pool = tc.alloc_tile_pool(name="oT", bufs=8)
        fstage_pool = tc.alloc_tile_pool(name="fstage", bufs=6)
        av_psum = tc.alloc_tile_pool(name="av_psum", bufs=2, space="PSUM")
        fin_psum = tc.alloc_tile_pool(name="fin_psum", bufs=2, space="PSUM")

        attnT = attnT_pool.tile([128, H, 4, N], BF16, tag="attnT")
        nc.sync.dma_start(
            out=attnT[:], in_=ag_out[:, :].rearrange("(h j p) i -> p h j i", p=128, j=4)
        )

        for rr in range(R):
            oTs = []
            for hp in range(4):
                ps = av_psum.tile([128, N], F32, tag="av")
                for jt in range(4):
                    for m in range(2):
                        h = 2 * hp + m
                        nc.tensor.matmul(
                            ps[m * 64:(m + 1) * 64, :],
                            v_tiles[(rr, jt)][:, h * D:(h + 1) * D],
                            attnT[:, h, jt, :],
                            start=(jt == 0),
                            stop=(jt == 3),
                            tile_position=(0, m * 64),
                            skip_group_check=True,
                        )
                oT = oT_pool.tile([128, N], F32R, tag="oT")
                nc.any.tensor_copy(oT[:], ps[:])
                oTs.append(oT)
            for ic in range(4):
                psf = fin_psum.tile([128, DIM], F32, tag="fin")
                for kc in range(4):
                    nc.tensor.matmul(
                        psf[:],
                        oTs[kc][:, ic * 128:(ic + 1) * 128],
                        wo_r[:, kc, :],
                        start=(kc == 0),
                        stop=(kc == 3),
                    )
                fst = fstage_pool.tile([128, DIM], F32, tag="fst")
                nc.any.tensor_copy(fst[:], psf[:])
                row0 = rr * N + ic * 128
                nc.sync.dma_start(out=out_ext[row0:row0 + 128, :], in_=fst[:])

        fin_psum.release()
        av_psum.release()
        fstage_pool.release()
        oT_pool.release()
        attnT_pool.release()
        xT_pool_released = True  # released above, before smax
        v_pool.release()
        consts.release()
        dram.release()

    if not nc.is_finalized():
        nc.finalize()
    return nc


_cache = {}


def _get_nc(scale: float):
    key = round(float(scale), 12)
    if key not in _cache:
        _cache[key] = build_nc(float(scale))
    return _cache[key]


def make_in_maps(x, Wq, Wkv, Wo):
    x = np.ascontiguousarray(np.asarray(x, dtype=np.float32)).reshape(CORES, RN, DIM)
    Wq = np.ascontiguousarray(np.asarray(Wq, dtype=np.float32))
    Wkv = np.asarray(Wkv, dtype=np.float32)
    Wk = np.ascontiguousarray(Wkv[:, :HD])
    Wv = np.ascontiguousarray(Wkv[:, HD:])
    Wo = np.ascontiguousarray(np.asarray(Wo, dtype=np.float32))
    return [
        {"x": x[c], "wq": Wq, "wk": Wk, "wv": Wv, "wo": Wo} for c in range(CORES)
    ]


def kernel(x, Wq, Wkv, Wo, bo, mask, tie_attn_dim):
    x = np.asarray(x)
    br, n, dim = x.shape
    r = int(tie_attn_dim)
    assert (br, n, dim) == (128, 512, 256) and r == 128, "kernel hardcodes shapes"
    mask = np.asarray(mask)
    assert mask.all(), "kernel assumes an all-valid mask"
    num_rows = float(mask.reshape(1, r, n).any(axis=-1).sum(axis=-1)[0])
    scale = (D ** -0.5) * (num_rows ** -0.5)

    nc = _get_nc(scale)
    in_maps = make_in_maps(x, Wq, Wkv, Wo)
    res = bass_utils.run_bass_kernel_spmd(nc, in_maps, core_ids=list(range(CORES)))
    out = np.concatenate([m["out"] for m in res.results], axis=0)
    out = out.reshape(br, n, dim)
    bo = np.asarray(bo, dtype=np.float32)
    if bo.any():
        out = out + bo
    return np.ascontiguousarray(out.astype(np.float32))


# revision 11
# speedup vs baseline: 1.1059x; 1.1059x over previous
"""Tied-row (MSA) attention, sharded over 8 TRN2 NeuronCores.

Reference computation (b=1, r=128 MSA rows, n=512, 8 heads x 64):
    q, k, v = x @ Wq, x @ Wk, x @ Wv          per-row projections
    dots[h,i,j] = sum_{r,d} q[r,h,i,d] k[r,h,j,d] * scale / sqrt(num_rows)
    attn = softmax_j(dots)                     shared across rows
    out[r,i] = (sum_j attn[h,i,j] v[r,h,j,d]) @ Wo + bo

Sharding: MSA-row axis r split 16-per-core.  Each core computes its partial
logits (reduction over its local r).  Partials are summed with bf16
ReduceScatters, one per head-pair, pipelined behind the next head-pair's
matmuls; each core softmaxes + transposes the [128 x 512] logit chunk it
owns and AllGathers it back as attn^T.  Everything else (projections,
attn @ v, output projection) is core-local.

x^T / v / attn are bf16 (SBUF capacity + collective size); q^T/k^T and
out^T are float32r (full-rate fp32 matmuls); accumulation is fp32 in PSUM.
"""

import numpy as np

import concourse.bacc as bacc
import concourse.bass as bass
import concourse.mybir as mybir
import concourse.tile as tile
from concourse import bass_utils
from concourse.masks import make_identity

CORES = 8
R = 16          # MSA rows per core
N = 512         # sequence length
DIM = 256       # model dim
H = 8           # heads
D = 64          # head dim
HD = H * D      # 512
RN = R * N      # 8192 token-rows per core

F32 = mybir.dt.float32
F32R = mybir.dt.float32r
BF16 = mybir.dt.bfloat16

RG = [list(range(CORES))]


def build_nc(scale: float):
    nc = bacc.Bacc(None, target_bir_lowering=False, debug=False)

    x_ext = nc.declare_dram_parameter("x", [RN, DIM], F32, isOutput=False)
    wq_ext = nc.declare_dram_parameter("wq", [DIM, HD], F32, isOutput=False)
    wk_ext = nc.declare_dram_parameter("wk", [DIM, HD], F32, isOutput=False)
    wv_ext = nc.declare_dram_parameter("wv", [DIM, HD], F32, isOutput=False)
    wo_ext = nc.declare_dram_parameter("wo", [HD, DIM], F32, isOutput=False)
    out_ext = nc.declare_dram_parameter("out", [RN, DIM], F32, isOutput=True)

    # alternate PSUM->SBUF copies between DVE and ScalarE so neither gates
    # PSUM-bank recycling
    _cp = [0]

    def cp(out, in_):
        if _cp[0] % 2 == 0:
            nc.vector.tensor_copy(out, in_)
        else:
            nc.scalar.copy(out, in_)
        _cp[0] += 1

    with tile.TileContext(nc) as tc:
        # ---- DRAM bounce buffers: one RS + one AG per head-pair ----
        dram = tc.alloc_tile_pool(name="dram", bufs=1, space="DRAM")
        rs_in = [dram.tile([2 * N, N], BF16, tag=f"rs_in{hp}", name=f"rs_in{hp}") for hp in range(4)]
        rs_out = [dram.tile([128, N], BF16, tag=f"rs_out{hp}", name=f"rs_out{hp}") for hp in range(4)]
        ag_in = [dram.tile([N, 128], BF16, tag=f"ag_in{hp}", name=f"ag_in{hp}") for hp in range(4)]
        ag_out = [
            dram.tile([CORES * N, 128], BF16, tag=f"ag_out{hp}", name=f"ag_out{hp}", addr_space="Shared")
            for hp in range(4)
        ]

        # ---- constants ----
        consts = tc.alloc_tile_pool(name="consts", bufs=1)
        wq_sb = consts.tile([128, 2, HD], BF16, tag="wq")
        wk_sb = consts.tile([128, 2, HD], BF16, tag="wk")
        wv_sb = consts.tile([128, 2, HD], BF16, tag="wv")
        wo_r = consts.tile([128, 4, DIM], F32R, tag="wor")
        id32 = consts.tile([128, 128], F32, tag="id32")
        idbf = consts.tile([128, 128], BF16, tag="idbf")
        wstage = tc.alloc_tile_pool(name="wstage", bufs=2)
        for wext, wsb in ((wq_ext, wq_sb), (wk_ext, wk_sb), (wv_ext, wv_sb)):
            wf = wstage.tile([128, 2, HD], F32, tag="wf")
            nc.sync.dma_start(
                out=wf[:], in_=wext[:, :].rearrange("(k p) n -> p k n", p=128)
            )
            nc.any.tensor_copy(wsb[:], wf[:])
        wof = wstage.tile([128, 4, DIM], F32, tag="wf")
        nc.sync.dma_start(
            out=wof[:], in_=wo_ext[:, :].rearrange("(k p) n -> p k n", p=128)
        )
        nc.any.tensor_copy(wo_r[:], wof[:])
        make_identity(nc, id32[:])
        make_identity(nc, idbf[:])
        wstage.release()

        # v tiles live from the v-projection until attn @ v; the pool outlives
        # xT, so allocate it first (pool releases are LIFO)
        v_pool = tc.alloc_tile_pool(name="v", bufs=R * 4)
        xT_pool = tc.alloc_tile_pool(name="xT", bufs=1)
        xT = xT_pool.tile([128, 2, RN], BF16, tag="xT")

        proj_psum = tc.alloc_tile_pool(name="proj_psum", bufs=3, space="PSUM")
        dots_psum = tc.alloc_tile_pool(name="dots_psum", bufs=3, space="PSUM")
        xp_psum = tc.alloc_tile_pool(name="xp_psum", bufs=2, space="PSUM")

        # ---- load x and transpose to x^T [dim(2x128), rn] (bf16) ----
        xrow_pool = tc.alloc_tile_pool(name="xrow", bufs=4)
        for c in range(RN // 128):
            xr = xrow_pool.tile([128, DIM], F32, tag="xr")
            nc.sync.dma_start(out=xr[:], in_=x_ext[c * 128:(c + 1) * 128, :])
            for kc in range(2):
                pt = xp_psum.tile([128, 128], F32, tag="xp")
                nc.tensor.transpose(pt[:], xr[:, kc * 128:(kc + 1) * 128], id32[:])
                cp(xT[:, kc, c * 128:(c + 1) * 128], pt[:])
        xrow_pool.release()
        xp_psum.release()
        atp_psum = tc.alloc_tile_pool(name="atp_psum", bufs=2, space="PSUM")

        qkT_pool = tc.alloc_tile_pool(name="qkT", bufs=1)
        dstage_pool = tc.alloc_tile_pool(name="dstage", bufs=6)
        smax_pool = tc.alloc_tile_pool(name="smax", bufs=2)
        attnT_pool_holder = []

        def softmax_block(hp):
            """exp/normalize/transpose the owned 128x512 chunk of RS #hp,
            then AllGather the transposed chunk."""
            zt = smax_pool.tile([128, N], BF16, tag="zt")
            nc.sync.dma_start(out=zt[:], in_=rs_out[hp][:, :])
            att_f = smax_pool.tile([128, N], F32, tag="att_f")
            sums = smax_pool.tile([128, 1], F32, tag="sums")
            nc.scalar.activation(
                att_f[:],
                zt[:],
                mybir.ActivationFunctionType.Exp,
                scale=scale,
                accum_out=sums[:],
            )
            recip = smax_pool.tile([128, 1], F32, tag="recip")
            nc.vector.reciprocal(recip[:], sums[:])
            abf = smax_pool.tile([128, N], BF16, tag="abf")
            nc.vector.tensor_scalar_mul(abf[:], att_f[:], recip[:])
            for jc in range(4):
                pt = atp_psum.tile([128, 128], BF16, tag="atp")
                nc.tensor.transpose(pt[:], abf[:, jc * 128:(jc + 1) * 128], idbf[:])
                ast = smax_pool.tile([128, 128], BF16, tag="agst", bufs=4)
                cp(ast[:], pt[:])
                nc.sync.dma_start(out=ag_in[hp][jc * 128:(jc + 1) * 128, :], in_=ast[:])
            nc.gpsimd.collective_compute(
                "AllGather",
                mybir.AluOpType.bypass,
                replica_groups=RG,
                ins=[ag_in[hp][:, :].opt()],
                outs=[ag_out[hp][:, :].opt()],
            )

        for hp in range(4):
            qT = qkT_pool.tile([128, RN], F32R, tag="qT")
            kT = qkT_pool.tile([128, RN], F32R, tag="kT")
            for wsb, dstT in ((wq_sb, qT), (wk_sb, kT)):
                for ch in range(RN // N):
                    ps = proj_psum.tile([128, N], F32, tag="proj")
                    for kc in range(2):
                        nc.tensor.matmul(
                            ps[:],
                            wsb[:, kc, hp * 128:(hp + 1) * 128],
                            xT[:, kc, ch * N:(ch + 1) * N],
                            start=(kc == 0),
                            stop=(kc == 1),
                        )
                    cp(dstT[:, ch * N:(ch + 1) * N], ps[:])

            # partial dots for the two heads of this pair; the even head uses
            # PE row-group 0-63, the odd head 64-127 (concurrent row tiles)
            for ic in range(4):
                pe_ = dots_psum.tile([128, N], F32, tag="dots")
                po_ = dots_psum.tile([128, N], F32, tag="dots")
                for rr in range(R):
                    base = rr * N
                    isl = slice(base + ic * 128, base + ic * 128 + 128)
                    jsl = slice(base, base + N)
                    nc.tensor.matmul(
                        pe_[:],
                        qT[0:64, isl],
                        kT[0:64, jsl],
                        start=(rr == 0),
                        stop=(rr == R - 1),
                        skip_group_check=True,
                    )
                    nc.tensor.matmul(
                        po_[:],
                        qT[64:128, isl],
                        kT[64:128, jsl],
                        start=(rr == 0),
                        stop=(rr == R - 1),
                        skip_group_check=True,
                    )
                for m, ps in ((0, pe_), (1, po_)):
                    st = dstage_pool.tile([128, N], BF16, tag="dstage")
                    cp(st[:], ps[:])
                    row0 = m * N + ic * 128
                    nc.sync.dma_start(out=rs_in[hp][row0:row0 + 128, :], in_=st[:])

            nc.gpsimd.collective_compute(
                "ReduceScatter",
                mybir.AluOpType.add,
                replica_groups=RG,
                ins=[rs_in[hp][:, :].opt()],
                outs=[rs_out[hp][:, :].opt()],
            )
            if hp >= 1:
                # softmax of the previous pair's chunk: its RS completed while
                # this pair's matmuls ran, so the PE transposes don't stall
                softmax_block(hp - 1)

        # ---- v projection (overlaps the last RS; last use of xT) ----
        v_tiles = {}
        for rr in range(R):
            for jc in range(4):
                ps = proj_psum.tile([128, N], F32, tag="proj")
                for kc in range(2):
                    nc.tensor.matmul(
                        ps[:],
                        xT[:, kc, rr * N + jc * 128:rr * N + jc * 128 + 128],
                        wv_sb[:, kc, :],
                        start=(kc == 0),
                        stop=(kc == 1),
                    )
                vt = v_pool.tile([128, HD], BF16, tag="v")
                cp(vt[:], ps[:])
                v_tiles[(rr, jc)] = vt

        softmax_block(3)

        smax_pool.release()
        dstage_pool.release()
        qkT_pool.release()
        xT_pool.release()
        atp_psum.release()
        dots_psum.release()
        proj_psum.release()

        # ---- attn^T @ v -> out^T, then out @ Wo ----
        attnT_pool = tc.alloc_tile_pool(name="attnT", bufs=1)
        oT_pool = tc.alloc_tile_pool(name="oT", bufs=8)
        fstage_pool = tc.alloc_tile_pool(name="fstage", bufs=6)
        av_psum = tc.alloc_tile_pool(name="av_psum", bufs=3, space="PSUM")
        fin_psum = tc.alloc_tile_pool(name="fin_psum", bufs=3, space="PSUM")

        attnT = attnT_pool.tile([128, H, 4, N], BF16, tag="attnT")
        for hp in range(4):
            for m in range(2):
                for iq in range(4):
                    blk = (m * 4 + iq) * N
                    nc.sync.dma_start(
                        out=attnT[:, 2 * hp + m, :, iq * 128:(iq + 1) * 128],
                        in_=ag_out[hp][blk:blk + N, :].rearrange(
                            "(jt p) i2 -> p jt i2", p=128
                        ),
                    )

        for rr in range(R):
            oTs = []
            for hp in range(4):
                ps = av_psum.tile([128, N], F32, tag="av")
                for jt in range(4):
                    for m in range(2):
                        h = 2 * hp + m
                        nc.tensor.matmul(
                            ps[m * 64:(m + 1) * 64, :],
                            v_tiles[(rr, jt)][:, h * D:(h + 1) * D],
                            attnT[:, h, jt, :],
                            start=(jt == 0),
                            stop=(jt == 3),
                            tile_position=(0, m * 64),
                            skip_group_check=True,
                        )
                oT = oT_pool.tile([128, N], F32R, tag="oT")
                cp(oT[:], ps[:])
                oTs.append(oT)
            for ic in range(4):
                psf = fin_psum.tile([128, DIM], F32, tag="fin")
                for kc in range(4):
                    nc.tensor.matmul(
                        psf[:],
                        oTs[kc][:, ic * 128:(ic + 1) * 128],
                        wo_r[:, kc, :],
                        start=(kc == 0),
                        stop=(kc == 3),
                    )
                fst = fstage_pool.tile([128, DIM], F32, tag="fst")
                cp(fst[:], psf[:])
                row0 = rr * N + ic * 128
                nc.sync.dma_start(out=out_ext[row0:row0 + 128, :], in_=fst[:])

        fin_psum.release()
        av_psum.release()
        fstage_pool.release()
        oT_pool.release()
        attnT_pool.release()
        v_pool.release()
        consts.release()
        dram.release()

    if not nc.is_finalized():
        nc.finalize()
    return nc


_cache = {}


def _get_nc(scale: float):
    key = round(float(scale), 12)
    if key not in _cache:
        _cache[key] = build_nc(float(scale))
    return _cache[key]


def make_in_maps(x, Wq, Wkv, Wo):
    x = np.ascontiguousarray(np.asarray(x, dtype=np.float32)).reshape(CORES, RN, DIM)
    Wq = np.ascontiguousarray(np.asarray(Wq, dtype=np.float32))
    Wkv = np.asarray(Wkv, dtype=np.float32)
    Wk = np.ascontiguousarray(Wkv[:, :HD])
    Wv = np.ascontiguousarray(Wkv[:, HD:])
    Wo = np.ascontiguousarray(np.asarray(Wo, dtype=np.float32))
    return [
        {"x": x[c], "wq": Wq, "wk": Wk, "wv": Wv, "wo": Wo} for c in range(CORES)
    ]


def kernel(x, Wq, Wkv, Wo, bo, mask, tie_attn_dim):
    x = np.asarray(x)
    br, n, dim = x.shape
    r = int(tie_attn_dim)
    assert (br, n, dim) == (128, 512, 256) and r == 128, "kernel hardcodes shapes"
    mask = np.asarray(mask)
    assert mask.all(), "kernel assumes an all-valid mask"
    num_rows = float(mask.reshape(1, r, n).any(axis=-1).sum(axis=-1)[0])
    scale = (D ** -0.5) * (num_rows ** -0.5)

    nc = _get_nc(scale)
    in_maps = make_in_maps(x, Wq, Wkv, Wo)
    res = bass_utils.run_bass_kernel_spmd(nc, in_maps, core_ids=list(range(CORES)))
    out = np.concatenate([m["out"] for m in res.results], axis=0)
    out = out.reshape(br, n, dim)
    bo = np.asarray(bo, dtype=np.float32)
    if bo.any():
        out = out + bo
    return np.ascontiguousarray(out.astype(np.float32))


# revision 12
# speedup vs baseline: 1.2325x; 1.1145x over previous
"""Tied-row (MSA) attention, sharded over 8 TRN2 NeuronCores.

Reference computation (b=1, r=128 MSA rows, n=512, 8 heads x 64):
    q, k, v = x @ Wq, x @ Wk, x @ Wv          per-row projections
    dots[h,i,j] = sum_{r,d} q[r,h,i,d] k[r,h,j,d] * scale / sqrt(num_rows)
    attn = softmax_j(dots)                     shared across rows
    out[r,i] = (sum_j attn[h,i,j] v[r,h,j,d]) @ Wo + bo

Sharding: MSA-row axis r split 16-per-core.  Each core computes its partial
logits (reduction over its local r).  Partials are summed with bf16
ReduceScatters, one per head-pair, pipelined two-deep behind the following
head-pairs' matmuls; each core softmaxes + transposes the [128 x 512] logit
chunk it owns and AllGathers it back as attn^T.  Everything else
(projections, attn @ v, output projection) is core-local.

x^T / v / attn are bf16 (SBUF capacity + collective size); q^T/k^T and
out^T are float32r (full-rate fp32 matmuls); accumulation is fp32 in PSUM.
"""

import numpy as np

import concourse.bacc as bacc
import concourse.bass as bass
import concourse.mybir as mybir
import concourse.tile as tile
from concourse import bass_utils
from concourse.masks import make_identity

CORES = 8
R = 16          # MSA rows per core
N = 512         # sequence length
DIM = 256       # model dim
H = 8           # heads
D = 64          # head dim
HD = H * D      # 512
RN = R * N      # 8192 token-rows per core

F32 = mybir.dt.float32
F32R = mybir.dt.float32r
BF16 = mybir.dt.bfloat16

RG = [list(range(CORES))]


def build_nc(scale: float):
    nc = bacc.Bacc(None, target_bir_lowering=False, debug=False)

    x_ext = nc.declare_dram_parameter("x", [RN, DIM], F32, isOutput=False)
    wq_ext = nc.declare_dram_parameter("wq", [DIM, HD], F32, isOutput=False)
    wk_ext = nc.declare_dram_parameter("wk", [DIM, HD], F32, isOutput=False)
    wv_ext = nc.declare_dram_parameter("wv", [DIM, HD], F32, isOutput=False)
    wo_ext = nc.declare_dram_parameter("wo", [HD, DIM], F32, isOutput=False)
    out_ext = nc.declare_dram_parameter("out", [RN, DIM], F32, isOutput=True)

    # alternate PSUM->SBUF copies between DVE and ScalarE so neither gates
    # PSUM-bank recycling
    _cp = [0]

    def cp(out, in_):
        if _cp[0] % 2 == 0:
            nc.vector.tensor_copy(out, in_)
        else:
            nc.scalar.copy(out, in_)
        _cp[0] += 1

    with tile.TileContext(nc) as tc:
        # ---- DRAM bounce buffers: one RS + one AG per head-pair ----
        dram = tc.alloc_tile_pool(name="dram", bufs=1, space="DRAM")
        rs_in = [dram.tile([2 * N, N], BF16, tag=f"rs_in{hp}", name=f"rs_in{hp}") for hp in range(4)]
        rs_out = [dram.tile([128, N], BF16, tag=f"rs_out{hp}", name=f"rs_out{hp}") for hp in range(4)]
        ag_in = [dram.tile([N, 128], BF16, tag=f"ag_in{hp}", name=f"ag_in{hp}") for hp in range(4)]
        ag_out = [
            dram.tile([CORES * N, 128], BF16, tag=f"ag_out{hp}", name=f"ag_out{hp}", addr_space="Shared")
            for hp in range(4)
        ]

        # ---- constants ----
        consts = tc.alloc_tile_pool(name="consts", bufs=1)
        wq_sb = consts.tile([128, 2, HD], BF16, tag="wq")
        wk_sb = consts.tile([128, 2, HD], BF16, tag="wk")
        wv_sb = consts.tile([128, 2, HD], BF16, tag="wv")
        wo_r = consts.tile([128, 4, DIM], F32R, tag="wor")
        id32 = consts.tile([128, 128], F32, tag="id32")
        idbf = consts.tile([128, 128], BF16, tag="idbf")
        wstage = tc.alloc_tile_pool(name="wstage", bufs=2)
        for wext, wsb in ((wq_ext, wq_sb), (wk_ext, wk_sb), (wv_ext, wv_sb)):
            wf = wstage.tile([128, 2, HD], F32, tag="wf")
            nc.sync.dma_start(
                out=wf[:], in_=wext[:, :].rearrange("(k p) n -> p k n", p=128)
            )
            nc.any.tensor_copy(wsb[:], wf[:])
        wof = wstage.tile([128, 4, DIM], F32, tag="wf")
        nc.sync.dma_start(
            out=wof[:], in_=wo_ext[:, :].rearrange("(k p) n -> p k n", p=128)
        )
        nc.any.tensor_copy(wo_r[:], wof[:])
        make_identity(nc, id32[:])
        make_identity(nc, idbf[:])
        wstage.release()

        # v tiles live from the v-projection until attn @ v; the pool outlives
        # everything else transient -> allocate early (pool releases are LIFO)
        v_pool = tc.alloc_tile_pool(name="v", bufs=R * 4)
        xT_pool = tc.alloc_tile_pool(name="xT", bufs=1)
        xT = xT_pool.tile([128, 2, RN], BF16, tag="xT")

        proj_psum = tc.alloc_tile_pool(name="proj_psum", bufs=3, space="PSUM")
        dots_psum = tc.alloc_tile_pool(name="dots_psum", bufs=3, space="PSUM")
        xp_psum = tc.alloc_tile_pool(name="xp_psum", bufs=2, space="PSUM")

        # ---- load x and transpose to x^T [dim(2x128), rn] (bf16) ----
        # 4 PE transposes batched per PSUM bank -> one [128,512] copy out
        xrow_pool = tc.alloc_tile_pool(name="xrow", bufs=8)
        for c4 in range(RN // N):
            xrs = []
            for j in range(4):
                c = c4 * 4 + j
                xr = xrow_pool.tile([128, DIM], F32, tag="xr")
                nc.sync.dma_start(out=xr[:], in_=x_ext[c * 128:(c + 1) * 128, :])
                xrs.append(xr)
            for kc in range(2):
                pt = xp_psum.tile([128, N], F32, tag="xp")
                for j in range(4):
                    nc.tensor.transpose(
                        pt[:, j * 128:(j + 1) * 128],
                        xrs[j][:, kc * 128:(kc + 1) * 128],
                        id32[:],
                    )
                cp(xT[:, kc, c4 * N:(c4 + 1) * N], pt[:])
        xrow_pool.release()
        xp_psum.release()
        atp_psum = tc.alloc_tile_pool(name="atp_psum", bufs=2, space="PSUM")

        dstage_pool = tc.alloc_tile_pool(name="dstage", bufs=4)
        smax_pool = tc.alloc_tile_pool(name="smax", bufs=2)
        qkT_pool = tc.alloc_tile_pool(name="qkT", bufs=1)

        def softmax_block(hp):
            """exp/normalize/transpose the owned 128x512 chunk of RS #hp,
            then AllGather the transposed chunk."""
            zt = smax_pool.tile([128, N], BF16, tag="zt")
            nc.sync.dma_start(out=zt[:], in_=rs_out[hp][:, :])
            att_f = smax_pool.tile([128, N], F32, tag="att_f")
            sums = smax_pool.tile([128, 1], F32, tag="sums")
            nc.scalar.activation(
                att_f[:],
                zt[:],
                mybir.ActivationFunctionType.Exp,
                scale=scale,
                accum_out=sums[:],
            )
            recip = smax_pool.tile([128, 1], F32, tag="recip")
            nc.vector.reciprocal(recip[:], sums[:])
            abf = smax_pool.tile([128, N], BF16, tag="abf")
            nc.vector.tensor_scalar_mul(abf[:], att_f[:], recip[:])
            pt = atp_psum.tile([128, N], BF16, tag="atp")
            for jc in range(4):
                nc.tensor.transpose(
                    pt[:, jc * 128:(jc + 1) * 128],
                    abf[:, jc * 128:(jc + 1) * 128],
                    idbf[:],
                )
            ast = smax_pool.tile([128, N], BF16, tag="agst")
            cp(ast[:], pt[:])
            for jc in range(4):
                nc.sync.dma_start(
                    out=ag_in[hp][jc * 128:(jc + 1) * 128, :],
                    in_=ast[:, jc * 128:(jc + 1) * 128],
                )
            nc.gpsimd.collective_compute(
                "AllGather",
                mybir.AluOpType.bypass,
                replica_groups=RG,
                ins=[ag_in[hp][:, :].opt()],
                outs=[ag_out[hp][:, :].opt()],
            )

        for hp in range(4):
            qT = qkT_pool.tile([128, RN], F32R, tag="qT")
            kT = qkT_pool.tile([128, RN], F32R, tag="kT")
            for wsb, dstT in ((wq_sb, qT), (wk_sb, kT)):
                for ch in range(RN // N):
                    ps = proj_psum.tile([128, N], F32, tag="proj")
                    for kc in range(2):
                        nc.tensor.matmul(
                            ps[:],
                            wsb[:, kc, hp * 128:(hp + 1) * 128],
                            xT[:, kc, ch * N:(ch + 1) * N],
                            start=(kc == 0),
                            stop=(kc == 1),
                        )
                    cp(dstT[:, ch * N:(ch + 1) * N], ps[:])

            # partial dots for the two heads of this pair; the even head uses
            # PE row-group 0-63, the odd head 64-127 (concurrent row tiles)
            for ic in range(4):
                pe_ = dots_psum.tile([128, N], F32, tag="dots")
                po_ = dots_psum.tile([128, N], F32, tag="dots")
                for rr in range(R):
                    base = rr * N
                    isl = slice(base + ic * 128, base + ic * 128 + 128)
                    jsl = slice(base, base + N)
                    nc.tensor.matmul(
                        pe_[:],
                        qT[0:64, isl],
                        kT[0:64, jsl],
                        start=(rr == 0),
                        stop=(rr == R - 1),
                        skip_group_check=True,
                    )
                    nc.tensor.matmul(
                        po_[:],
                        qT[64:128, isl],
                        kT[64:128, jsl],
                        start=(rr == 0),
                        stop=(rr == R - 1),
                        skip_group_check=True,
                    )
                for m, ps in ((0, pe_), (1, po_)):
                    st = dstage_pool.tile([128, N], BF16, tag="dstage")
                    cp(st[:], ps[:])
                    row0 = m * N + ic * 128
                    nc.sync.dma_start(out=rs_in[hp][row0:row0 + 128, :], in_=st[:])

            nc.gpsimd.collective_compute(
                "ReduceScatter",
                mybir.AluOpType.add,
                replica_groups=RG,
                ins=[rs_in[hp][:, :].opt()],
                outs=[rs_out[hp][:, :].opt()],
            )
            if hp >= 2:
                # softmax of the pair two back: its RS completed while the
                # intervening pairs' matmuls ran, so PE never stalls on it
                softmax_block(hp - 2)

        qkT_pool.release()

        # ---- v projection (overlaps the last two RS; reads xT) ----
        v_tiles = {}
        for rr in range(R):
            for jc in range(4):
                ps = proj_psum.tile([128, N], F32, tag="proj")
                for kc in range(2):
                    nc.tensor.matmul(
                        ps[:],
                        xT[:, kc, rr * N + jc * 128:rr * N + jc * 128 + 128],
                        wv_sb[:, kc, :],
                        start=(kc == 0),
                        stop=(kc == 1),
                    )
                vt = v_pool.tile([128, HD], BF16, tag="v")
                cp(vt[:], ps[:])
                v_tiles[(rr, jc)] = vt

        softmax_block(2)
        softmax_block(3)

        smax_pool.release()
        dstage_pool.release()
        atp_psum.release()
        dots_psum.release()
        proj_psum.release()

        # ---- attn^T @ v -> out^T, then out @ Wo ----
        # attnT/oT/fstage reuse the SBUF freed by qkT/dstage/smax (xT stays
        # allocated, so no WAR against the v-projection reads of xT)
        attnT_pool = tc.alloc_tile_pool(name="attnT", bufs=1)
        oT_pool = tc.alloc_tile_pool(name="oT", bufs=8)
        fstage_pool = tc.alloc_tile_pool(name="fstage", bufs=6)
        av_psum = tc.alloc_tile_pool(name="av_psum", bufs=3, space="PSUM")
        fin_psum = tc.alloc_tile_pool(name="fin_psum", bufs=3, space="PSUM")

        attnT = attnT_pool.tile([128, H, 4, N], BF16, tag="attnT")
        for hp in range(4):
            for m in range(2):
                for iq in range(4):
                    blk = (m * 4 + iq) * N
                    nc.sync.dma_start(
                        out=attnT[:, 2 * hp + m, :, iq * 128:(iq + 1) * 128],
                        in_=ag_out[hp][blk:blk + N, :].rearrange(
                            "(jt p) i2 -> p jt i2", p=128
                        ),
                    )

        for rr in range(R):
            oTs = []
            for hp in range(4):
                ps = av_psum.tile([128, N], F32, tag="av")
                for jt in range(4):
                    for m in range(2):
                        h = 2 * hp + m
                        nc.tensor.matmul(
                            ps[m * 64:(m + 1) * 64, :],
                            v_tiles[(rr, jt)][:, h * D:(h + 1) * D],
                            attnT[:, h, jt, :],
                            start=(jt == 0),
                            stop=(jt == 3),
                            tile_position=(0, m * 64),
                            skip_group_check=True,
                        )
                oT = oT_pool.tile([128, N], F32R, tag="oT")
                cp(oT[:], ps[:])
                oTs.append(oT)
            for ic in range(4):
                psf = fin_psum.tile([128, DIM], F32, tag="fin")
                for kc in range(4):
                    nc.tensor.matmul(
                        psf[:],
                        oTs[kc][:, ic * 128:(ic + 1) * 128],
                        wo_r[:, kc, :],
                        start=(kc == 0),
                        stop=(kc == 3),
                    )
                fst = fstage_pool.tile([128, DIM], F32, tag="fst")
                cp(fst[:], psf[:])
                row0 = rr * N + ic * 128
                nc.sync.dma_start(out=out_ext[row0:row0 + 128, :], in_=fst[:])

        fin_psum.release()
        av_psum.release()
        fstage_pool.release()
        oT_pool.release()
        attnT_pool.release()
        xT_pool.release()
        v_pool.release()
        consts.release()
        dram.release()

    if not nc.is_finalized():
        nc.finalize()
    return nc


_cache = {}


def _get_nc(scale: float):
    key = round(float(scale), 12)
    if key not in _cache:
        _cache[key] = build_nc(float(scale))
    return _cache[key]


def make_in_maps(x, Wq, Wkv, Wo):
    x = np.ascontiguousarray(np.asarray(x, dtype=np.float32)).reshape(CORES, RN, DIM)
    Wq = np.ascontiguousarray(np.asarray(Wq, dtype=np.float32))
    Wkv = np.asarray(Wkv, dtype=np.float32)
    Wk = np.ascontiguousarray(Wkv[:, :HD])
    Wv = np.ascontiguousarray(Wkv[:, HD:])
    Wo = np.ascontiguousarray(np.asarray(Wo, dtype=np.float32))
    return [
        {"x": x[c], "wq": Wq, "wk": Wk, "wv": Wv, "wo": Wo} for c in range(CORES)
    ]


def kernel(x, Wq, Wkv, Wo, bo, mask, tie_attn_dim):
    x = np.asarray(x)
    br, n, dim = x.shape
    r = int(tie_attn_dim)
    assert (br, n, dim) == (128, 512, 256) and r == 128, "kernel hardcodes shapes"
    mask = np.asarray(mask)
    assert mask.all(), "kernel assumes an all-valid mask"
    num_rows = float(mask.reshape(1, r, n).any(axis=-1).sum(axis=-1)[0])
    scale = (D ** -0.5) * (num_rows ** -0.5)

    nc = _get_nc(scale)
    in_maps = make_in_maps(x, Wq, Wkv, Wo)
    res = bass_utils.run_bass_kernel_spmd(nc, in_maps, core_ids=list(range(CORES)))
    out = np.concatenate([m["out"] for m in res.results], axis=0)
    out = out.reshape(br, n, dim)
    bo = np.asarray(bo, dtype=np.float32)
    if bo.any():
        out = out + bo
    return np.ascontiguousarray(out.astype(np.float32))
